# revision 15
# baseline (speedup 1.0000x reference)
"""Trainium2 Bass kernel for the RN (relation-network) module.

Math per batch b:
  Xe = emb[X[b]]                        (n=128 tokens, D=256)
  A = Xe @ W_l.T ; Bf = Xe @ W_r.T + (b_l + b_r)
  pooled[b] = sum_{i,j} relu(A[j] + Bf[i])
  out[b] = pooled[b] @ W_rn.T + n^2 * b_rn

The n^2 pairwise band dominates. Per unit (dc = chunk of 128 feature dims,
batch s), a [128 x 128] (i, j) tile of relu sums is produced in two pieces
into a shared bf16 buffer mrows[128p, i, j], then folded with one reduce:

  - DVE, one tensor_tensor(add) per unit at bf16 2x_1p: raw A_j + Bf_i for
    i in [AV, 128). Both operands present the innermost dim as adjacent
    PAIRS so every AP has a step-1 innermost dim (the broadcast of Bf along
    j is expressed via a materialized [Bf_i, Bf_i] pair table, middle dims
    use stride 0) -- that is what unlocks the 2x perf mode; a plain
    broadcast AP would fall back to 1x.
  - Act, activation(Relu, bias=Bf_i) rows for i in [0, AV): finished relu
    rows (Act applies bias+relu in one pass; it has no fast modes but is
    otherwise idle).
  - DVE, one tensor_scalar(op0=max(0), op1=add-reduce, accum_out) per unit
    at bf16 4x_2p over the whole mrows tile: applies relu to the raw rows
    (idempotent on Act's rows) and sums everything into pooled[dc][:, s].

Rejected alternatives (measured on the CoreSim cost model): per-row
tensor_scalar 189ns/row; scalar_tensor_tensor / tensor_tensor_reduce are
1x-only (134ns/row); Pool/GPSIMD cannot encode any tensor op besides copy
(walrus "Instruction engine check failed") and is kept off the critical
path. Two PSUM accumulation groups must not interleave within one bank
(wrong sums on HW). Cost-model time: 97.4us vs 162.6us for the v1 STT
kernel; DVE and Act both ~85-90% busy, so this is near the 2-engine floor
(DVE 1.28 elem/ns + Act 0.43 elem/ns over 16.8M pairwise elems/core).

Embedding gather + Xe transpose are host-side; inputs arrive as Xe^T bf16.
Sharding: batch data-parallel, 4 batches per core across 8 cores.
"""

import json

import numpy as np
import ml_dtypes

import concourse.bass as bass
import concourse.tile as tile
from concourse import mybir
from concourse.bass_utils import run_bass_kernel_spmd

B, SEQ, D, VOCAB = 32, 128, 256, 32000
NCORES = 8
BPC = B // NCORES        # batches per core
NTOK = BPC * SEQ         # tokens gathered per core
F32 = mybir.dt.float32
BF16 = mybir.dt.bfloat16

AV = 34                  # Act handles i in [0, AV); DVE TT i in [AV, 128)
AV_PROFILE = None        # optional per-unit AV override (list of 8)
POOL_BFREP = False       # build the bfrep pair table on Pool instead of DVE
SPLIT_XET = False        # DMA xet in two kc chunks for earlier matmul start
NMR = 3                  # mrows rotation depth

_NC_CACHE = {}


def _build_nc(for_sim=False):
    nc = bass.Bass()
    xet_d = nc.declare_dram_parameter("xet", [128, 2 * NTOK], BF16, isOutput=False)
    wlt_d = nc.declare_dram_parameter("wlt", [128, 2 * D], BF16, isOutput=False)
    wrt_d = nc.declare_dram_parameter("wrt", [128, 2 * D], BF16, isOutput=False)
    wrnt_d = nc.declare_dram_parameter("wrnt", [128, 2 * D], F32, isOutput=False)
    cst_d = nc.declare_dram_parameter("cst", [128, 4], F32, isOutput=False)
    out_d = nc.declare_dram_parameter("out", [D, BPC], F32, isOutput=True)

    AF = mybir.ActivationFunctionType
    OP = mybir.AluOpType
    IV = 128 - AV

    with tile.TileContext(nc) as tc:
        with (
            tc.tile_pool(name="sb", bufs=1) as sb,
            tc.tile_pool(name="ps", bufs=1, space=bass.MemorySpace.PSUM) as ps,
        ):
            a_ps = [ps.tile([128, NTOK], F32, tag=f"a{dc}", name=f"a{dc}") for dc in range(2)]
            b_ps = [ps.tile([128, NTOK], F32, tag=f"b{dc}", name=f"b{dc}") for dc in range(2)]
            o_ps2 = ps.tile([128, 2, BPC], F32, tag="o", name="o")
            o_ps = [o_ps2[:, mc, :] for mc in range(2)]
            warm = ps.tile([128, 1], F32, tag="warm", name="warm")

            wlt_sb = sb.tile([128, 2, D], BF16, tag="wlt", name="wlt")
            wrt_sb = sb.tile([128, 2, D], BF16, tag="wrt", name="wrt")
            wrnt_sb = sb.tile([128, 2, D], F32, tag="wrnt", name="wrnt")
            cst_sb = sb.tile([128, 4], F32, tag="cst", name="cst")
            xet = sb.tile([128, 2, NTOK], BF16, tag="xet", name="xet")
            a_bf = sb.tile([128, 2, NTOK], BF16, tag="a_bf", name="a_bf")
            bfull = sb.tile([128, 2, NTOK], F32, tag="bfull", name="bfull")
            bfrep = sb.tile([128, 2, NTOK, 2], BF16, tag="bfrep", name="bfrep")
            mrows = [sb.tile([128, 128, 128], BF16, tag=f"mr{k}", name=f"mr{k}")
                     for k in range(NMR)]
            uacc = sb.tile([128, 2, BPC], F32, tag="uacc", name="uacc")
            out_sb = [sb.tile([128, BPC], F32, tag=f"out{mc}", name=f"out{mc}") for mc in range(2)]

            sp = nc.sync
            with tc.high_priority():
                sp.dma_start(wlt_sb[:], wlt_d[:].rearrange("p (kc d) -> p kc d", kc=2))
                xet_src = xet_d[:].rearrange("p (kc t) -> p kc t", kc=2)
                if SPLIT_XET:
                    sp.dma_start(xet[:, 0, :], xet_src[:, 0, :])
                    sp.dma_start(xet[:, 1, :], xet_src[:, 1, :])
                else:
                    sp.dma_start(xet[:], xet_src)
                sp.dma_start(wrt_sb[:], wrt_d[:].rearrange("p (kc d) -> p kc d", kc=2))
                sp.dma_start(cst_sb[:], cst_d[:])
                sp.dma_start(wrnt_sb[:], wrnt_d[:].rearrange("p (kc d) -> p kc d", kc=2))

                # HW LDWEIGHTS takes at most 1 sync wait => dead PE dummies
                # absorb the weight DMA waits so real matmuls only wait on xet.
                nc.tensor.matmul(warm[0:1, 0:1], wlt_sb[:, 0, 0:1], wlt_sb[:, 0, 0:1], start=True, stop=True)
                nc.tensor.matmul(warm[0:1, 0:1], wrt_sb[:, 0, 0:1], wrt_sb[:, 0, 0:1], start=True, stop=True)
                nc.tensor.matmul(warm[0:1, 0:1], wrnt_sb[:, 0, 0:1], wrnt_sb[:, 0, 0:1], start=True, stop=True)

                # a_ps[dc][do, t] = sum_k Wl[do, k] XeT[k, t]; likewise b
                for dc in range(2):
                    cols = slice(dc * 128, (dc + 1) * 128)
                    for kc in range(2):
                        nc.tensor.matmul(a_ps[dc][:], wlt_sb[:, kc, cols], xet[:, kc, :],
                                         start=(kc == 0), stop=(kc == 1))
                    for kc in range(2):
                        nc.tensor.matmul(b_ps[dc][:], wrt_sb[:, kc, cols], xet[:, kc, :],
                                         start=(kc == 0), stop=(kc == 1))

                # Act: a_bf = bf16(A); bfull = B + blr (f32, the Relu-row bias)
                # DVE: bfrep = [Bf, Bf] bf16 pair table for the TT band
                for dc in range(2):
                    nc.scalar.copy(a_bf[:, dc, :], a_ps[dc][:])
                    nc.scalar.activation(
                        bfull[:, dc, :], b_ps[dc][:],
                        AF.Identity, bias=cst_sb[:, dc:dc + 1], scale=1.0)
                    bsrc = bfull[:, dc, :].unsqueeze(2).broadcast_to([128, NTOK, 2])
                    if POOL_BFREP:
                        nc.gpsimd.tensor_copy(bfrep[:, dc, :, :], bsrc)
                    else:
                        nc.vector.tensor_scalar(bfrep[:, dc, :, :], bsrc, 1.0, None, OP.mult)

            # ---- the pairwise band
            avs = AV_PROFILE or [AV] * 8
            for dc in range(2):
                for s in range(BPC):
                    u = dc * BPC + s
                    av_u = avs[u]
                    iv_u = 128 - av_u
                    mr = mrows[u % NMR]
                    seg = slice(s * 128, (s + 1) * 128)
                    a_seg = a_bf[:, dc, seg]
                    for i in range(av_u):
                        nc.scalar.activation(
                            mr[:, i, :], a_seg, AF.Relu,
                            bias=bfull[:, dc, s * 128 + i: s * 128 + i + 1], scale=1.0)
                    # raw A_j + Bf_i rows for i in [AV, 128), pair-packed APs
                    a_pair = (a_seg.rearrange("p (j2 jp) -> p j2 jp", jp=2)
                              .unsqueeze(1).broadcast_to([128, iv_u, 64, 2]))
                    b_pair = (bfrep[:, dc, s * 128 + av_u:(s + 1) * 128, :]
                              .unsqueeze(2).broadcast_to([128, iv_u, 64, 2]))
                    nc.vector.tensor_tensor(
                        mr[:, av_u:128, :].rearrange("p i (j2 jp) -> p i j2 jp", jp=2),
                        a_pair, b_pair, OP.add)
                    # relu + sum the whole unit (4x mode, in place)
                    flat = mr[:].rearrange("p i j -> p (i j)")
                    nc.vector.tensor_scalar(
                        flat, flat, 0.0, 0.0, OP.max, OP.add,
                        accum_out=uacc[:, dc, s:s + 1])

            # out[mo, s] = sum_do wrnt[do, mo] * pooled[do, s] + n^2 b_rn
            # keep each PSUM accumulation group contiguous (interleaving two
            # start/stop groups in one bank computes wrong sums on HW)
            for mc in range(2):
                cols = slice(mc * 128, (mc + 1) * 128)
                for dc in range(2):
                    nc.tensor.matmul(o_ps[mc][:], wrnt_sb[:, dc, cols], uacc[:, dc, :],
                                     start=(dc == 0), stop=(dc == 1))
                nc.scalar.activation(out_sb[mc][:], o_ps[mc][:], AF.Identity,
                                     bias=cst_sb[:, 2 + mc:3 + mc], scale=1.0)
                sp.dma_start(out_d[:][mc * 128:(mc + 1) * 128, :], out_sb[mc][:])

    if not for_sim:
        _strip_own_engine_waits(nc)
    return nc


def _strip_own_engine_waits(nc):
    # Engines retire their queue in order, so a wait on the engine's own
    # counting semaphore is always satisfied by program order; walrus codegen
    # only encodes one wait per instruction, so drop the redundant ones.
    orig = nc.to_json_bytes

    def patched():
        d = json.loads(orig())

        def walk(o):
            if isinstance(o, dict):
                yield o
                for v in o.values():
                    yield from walk(v)
            elif isinstance(o, list):
                for v in o:
                    yield from walk(v)

        for o in walk(d):
            if isinstance(o, dict) and "opcode" in o and "sync_info" in o:
                eng = o.get("engine")
                si = o["sync_info"] or {}
                ws = si.get("on_wait") or []
                if eng and len(ws) > 1:
                    own = eng + "_44"
                    kept = [w for w in ws if w.get("ant_name") != own]
                    if kept and len(kept) < len(ws):
                        si["on_wait"] = kept

        # any instruction still carrying >1 wait: prepend single-wait Drain
        # shims on the same in-order queue (AND of waits via program order)
        def fix_list(lst):
            out = []
            for ins in lst:
                if isinstance(ins, dict) and "opcode" in ins:
                    si = ins.get("sync_info") or {}
                    ws = si.get("on_wait") or []
                    if len(ws) > 1 and ins.get("engine"):
                        for i, w in enumerate(ws[:-1]):
                            out.append({
                                "debug": ins.get("debug", 0),
                                "engine": ins["engine"],
                                "ins": [], "is_reset_sema": False,
                                "name": f"{ins['name']}_w{i}",
                                "opcode": "Drain", "outs": [],
                                "sync_info": {"on_update": [], "on_wait": [w]},
                            })
                        si["on_wait"] = [ws[-1]]
                out.append(ins)
            lst[:] = out

        def walk_lists(o):
            if isinstance(o, dict):
                for v in o.values():
                    walk_lists(v)
            elif isinstance(o, list):
                if any(isinstance(x, dict) and "opcode" in x for x in o):
                    fix_list(o)
                else:
                    for v in o:
                        walk_lists(v)

        walk_lists(d)
        return json.dumps(d).encode()

    nc.to_json_bytes = patched


def _get_nc():
    if "nc" not in _NC_CACHE:
        _NC_CACHE["nc"] = _build_nc()
    return _NC_CACHE["nc"]


def _prep_inputs(X, emb, W_l, b_l, W_r, b_r, W_rn, b_rn):
    emb = np.asarray(emb, dtype=np.float32)

    # w*t_sb[p, kc, do] = W.T[kc*128+p, do]
    def chunked_T(W, dt):
        wt = np.asarray(W, dtype=np.float32).T.reshape(2, 128, D).transpose(1, 0, 2)
        return np.ascontiguousarray(wt.reshape(128, 2 * D).astype(dt))

    wlt = chunked_T(W_l, ml_dtypes.bfloat16)
    wrt = chunked_T(W_r, ml_dtypes.bfloat16)
    wrnt = chunked_T(W_rn, np.float32)
    blr = (np.asarray(b_l, dtype=np.float32) + np.asarray(b_r, dtype=np.float32))
    cst = np.zeros((128, 4), np.float32)
    cst[:, 0:2] = blr.reshape(2, 128).T
    cst[:, 2:4] = (float(SEQ * SEQ) * np.asarray(b_rn, dtype=np.float32)).reshape(2, 128).T

    Xi = np.asarray(X)[:, :SEQ].astype(np.int64)
    in_maps = []
    for c in range(NCORES):
        order = Xi[c * BPC:(c + 1) * BPC, :].reshape(-1)       # g = b_local*128 + j
        # xet[k, kc, t] = Xe[t, kc*128+k]
        xeT = emb[order].T.reshape(2, 128, NTOK).transpose(1, 0, 2)
        xeT = np.ascontiguousarray(xeT.reshape(128, 2 * NTOK).astype(ml_dtypes.bfloat16))
        in_maps.append({"xet": xeT, "wlt": wlt, "wrt": wrt,
                        "wrnt": wrnt, "cst": cst})
    return in_maps


def _run(inputs, trace=False):
    nc = _get_nc()
    in_maps = _prep_inputs(**inputs)
    res = run_bass_kernel_spmd(nc, in_maps, list(range(NCORES)), trace=trace)
    out = np.concatenate([np.asarray(r["out"]).T for r in res.results], axis=0)
    return out.astype(np.float32), res


def kernel(**inputs):
    out, _ = _run(inputs, trace=False)
    return out
